# revision 19
# baseline (speedup 1.0000x reference)
"""Trainium2 Bass kernel for nn_GAttention (gnn_message_passing).

Computation (per batch b):
    k  = einsum('cnt,c->nt', x[b], alpha)
    kG = k @ Gw
    S  = kG @ k.T                  # [N, N]
    att = softmax(S, axis=-1)      # rows
    out[b] = einsum('nm,cmt->cnt', att * adj, x[b])

Sharding: data-parallel over batch B=16 across 8 cores (2 batches/core).
adj/Gw/alpha replicated. No collectives.

Device-side layout strategy (per batch):
  - x loaded transposed as xT[m, (c,t)] tiles (m on partitions) via strided
    DMA; cast to bf16 for the aggregation matmul; k computed from the fp32
    tiles on DVE (alpha-weighted tree reduction over c).
  - scores computed TRANSPOSED: ST[m, n] = k[m] . kG[n] so that the
    softmax-masked weights come out in the [m, n] layout the aggregation
    matmul needs as its stationary operand (contraction over m).
  - softmax without max-subtraction (scores are O(+-15), exp is safe in
    fp32); denominator = column sums of exp(ST), computed with a
    ones-vector matmul trick accumulated into a single PSUM bank.
  - aggregation: out2[n, (c,t)] = sum_m wT[m, n] * xT[m, (c,t)] in bf16,
    PSUM fp32 accumulation over 16 m-tiles; evicted through ScalarE with
    per-partition 1/denom scale, stored back with strided DMA.
"""

import functools

import numpy as np

import concourse.bass as bass
import concourse.bacc as bacc
import concourse.mybir as mybir
import concourse.tile as tile
from concourse.bass_utils import run_bass_kernel_spmd
from concourse.masks import make_identity

# Problem shape (hardcoded per contract).
B, C, N, T = 16, 64, 2048, 24
NCORES = 8
BPC = B // NCORES            # batches per core
P = 128                      # partitions
CT = C * T                   # 1536
NT = N // P                  # 16 n/m tiles
NHALF = 1024                 # n processed in halves (SBUF budget for wT)
NTL = NHALF // P             # 8 n-tiles per half
F32 = mybir.dt.float32
F32R = mybir.dt.float32r     # fp32 storage, single-pass PE multiply
BF16 = mybir.dt.bfloat16


def ts(i, sz):
    return bass.ts(i, sz)


def _build_kernel_body(tc: tile.TileContext, x, adjt, gw, alpha, out, reps=1):
    nc = tc.nc
    ctx_pools = []

    def pool(name, bufs, space="SBUF"):
        p = tc.alloc_tile_pool(name=name, bufs=bufs, space=space)
        ctx_pools.append(p)
        return p

    singles = pool("singles", 1)
    adjp = pool("adjp", 4)           # streamed bf16 adjT tiles (+ prepass)
    xfp = pool("xf", 3)              # fp32 x staging (strided loads land here)
    xbp = pool("xb", 24)             # bf16 xT tiles (16/batch + prefetch)
    kp = pool("kp", 2)               # k [128, 16, 24] per batch
    ktp = pool("ktp", 1)             # kT [24, 2048] f32r per batch
    kgp = pool("kgp", 1)             # kGT [24, 2048] f32r per batch
    ep = pool("ep", 10)              # exp(ST) bf16 chunks
    wtp = pool("wtp", 20)            # wT half-tiles [128, 1024] bf16
    osbp = pool("osb", 2)            # output staging fp32
    scrp = pool("scr", 1)            # k-chain scratch (DVE-serial, 1 buf ok)
    rcp = pool("rcp", 2)             # reciprocal denominators
    drp = pool("dram", 1, space="DRAM")      # bf16 adjT scratch in HBM
    ps_st = pool("ps_st", 2, space="PSUM")   # scores / small matmuls
    ps_dn = pool("ps_dn", 1, space="PSUM")   # denominator bank
    ps_o = pool("ps_o", 4, space="PSUM")     # aggregation accumulators

    # --- one-time setup ---------------------------------------------------
    ident = singles.tile([P, P], F32)
    make_identity(nc, ident)

    alpha_rep = singles.tile([P, C], F32)
    nc.gpsimd.dma_start(
        out=alpha_rep,
        in_=bass.AP(tensor=alpha.tensor, offset=0, ap=[[0, P], [1, C]]),
    )

    gw_sb = singles.tile([T, T], F32R)
    nc.gpsimd.dma_start(out=gw_sb, in_=gw[:, :])

    # e_q stationaries for the denominator trick: column q is ones.
    eq_tiles = []
    for q in range(2):
        e_q = singles.tile([P, 2], BF16, name=f"eq{q}")
        nc.vector.memset(e_q, 0.0)
        nc.vector.memset(e_q[:, q : q + 1], 1.0)
        eq_tiles.append(e_q)

    # Prepass: adjT fp32 (HBM) -> bf16 scratch in HBM, read once, used 2x.
    adjt16 = drp.tile([N, N], BF16, name="adjt16")
    for mt in range(NT):
        t_ = adjp.tile([P, N], BF16, name="adjpre", tag="adj")
        nc.gpsimd.dma_start(out=t_, in_=adjt[ts(mt, P), :])
        nc.sync.dma_start(out=adjt16[ts(mt, P), :], in_=t_)

    # --- per batch --------------------------------------------------------
    for b in [bi for _ in range(reps) for bi in range(BPC)]:
        x_b = x[b].rearrange("c (mo p) t -> mo p c t", p=P)      # [16,128,C,T]
        out_b = out[b].rearrange("c (no p) t -> no p c t", p=P)  # [16,128,C,T]

        xb_tiles = []
        k_all = kp.tile([P, NT, T], F32, name="k_all")
        kt_sb = ktp.tile([T, N], F32R, name="kt")
        kgt_sb = kgp.tile([T, N], F32R, name="kgt")

        def load_tile(mt, x_b=x_b, k_all=k_all, kt_sb=kt_sb, xb_tiles=xb_tiles):
            """Strided xT load + k-chain + kT transpose + bf16 cast."""
            xf = xfp.tile([P, CT], F32, name="xf")
            xf3 = xf.rearrange("p (c t) -> p c t", t=T)
            # Split across both HWDGE rings (SP + ACT) so descriptor
            # generation for the 96B-chunk strided pattern runs in parallel.
            nc.sync.dma_start(out=xf3[:, : C // 2, :], in_=x_b[mt][:, : C // 2, :])
            nc.scalar.dma_start(out=xf3[:, C // 2 :, :], in_=x_b[mt][:, C // 2 :, :])

            # k[m, t] = sum_c alpha[c] * x[c, m, t] via scratch so the
            # k-chain does not block on xb slot availability (batch overlap).
            scr = scrp.tile([P, C, T], F32, name="scr")
            nc.vector.tensor_tensor(
                scr,
                xf3,
                alpha_rep[:, :, None].to_broadcast((P, C, T)),
                mybir.AluOpType.mult,
            )
            s = C // 2
            while s >= 1:
                nc.vector.tensor_add(
                    out=scr[:, :s, :], in0=scr[:, :s, :], in1=scr[:, s : 2 * s, :]
                )
                s //= 2
            nc.vector.tensor_copy(out=k_all[:, mt, :], in_=scr[:, 0, :])

            # kT via PE transpose, interleaved with the load stream.
            ps = ps_st.tile([P, 512], F32, name="st")
            nc.tensor.transpose(ps[:T, :P], k_all[:, mt, :], ident)
            nc.vector.tensor_copy(out=kt_sb[:, ts(mt, P)], in_=ps[:T, :P])

            xb_t = xbp.tile([P, CT], BF16, name="xb")
            nc.vector.tensor_copy(out=xb_t, in_=xf)
            xb_tiles.append(xb_t)

        def kgt_half(h, kt_sb=kt_sb, kgt_sb=kgt_sb):
            # kGT[s, n] = sum_t Gw[t, s] * kT[t, n] for this n-half
            for q4 in range(2):
                qg = h * 2 + q4
                ps = ps_st.tile([P, 512], F32, name="st")
                nc.tensor.matmul(
                    ps[:T, :512], gw_sb, kt_sb[:, ts(qg, 512)],
                    start=True, stop=True,
                )
                nc.vector.tensor_copy(out=kgt_sb[:, ts(qg, 512)], in_=ps[:T, :512])

        def phase1_tile(h, mt, dn, wt_tiles, kt_sb=kt_sb, kgt_sb=kgt_sb):
            """ST -> exp -> denominator MM -> masked wT for one m-tile."""
            adj_t = adjp.tile([P, NHALF], BF16, name="adjs", tag="adj")
            nc.sync.dma_start(
                out=adj_t, in_=adjt16[ts(mt, P), h * NHALF : (h + 1) * NHALF]
            )
            wt_t = wtp.tile([P, NHALF], BF16, name="wt")
            wt_tiles.append(wt_t)
            for q in range(2):
                nsl = slice(h * NHALF + q * 512, h * NHALF + (q + 1) * 512)
                st_t = ps_st.tile([P, 512], F32, name="st")
                nc.tensor.matmul(
                    st_t, kt_sb[:, ts(mt, P)], kgt_sb[:, nsl],
                    start=True, stop=True,
                )
                e_t = ep.tile([P, 512], BF16, name="e")
                nc.scalar.activation(
                    out=e_t, in_=st_t, func=mybir.ActivationFunctionType.Exp
                )
                nc.tensor.matmul(
                    dn, eq_tiles[q], e_t,
                    start=(mt == 0 and q == 0),
                    stop=(mt == NT - 1 and q == 1),
                )
                nc.vector.tensor_mul(
                    out=wt_t[:, ts(q, 512)], in0=e_t, in1=adj_t[:, ts(q, 512)]
                )

        def recip_half(dn):
            # recipT[p, j] = 1 / denom[n = h*1024 + j*128 + p], j = 0..7
            recip_sb = rcp.tile([2, 512], F32, name="recip")
            nc.vector.reciprocal(out=recip_sb, in_=dn)
            recip_t = rcp.tile([P, NTL], F32, name="recipt")
            for j in range(NTL):
                q, j4 = divmod(j, 4)
                nc.sync.dma_start(
                    out=recip_t[:, j : j + 1],
                    in_=recip_sb[q : q + 1, j4 * P : (j4 + 1) * P],
                )
            return recip_t

        def agg_half(h, wt_tiles, recip_t, out_b=out_b, xb_tiles=xb_tiles):
            for ntl in range(NTL):
                nt_g = h * NTL + ntl
                o_ts = [ps_o.tile([P, 512], F32, name="o") for _ in range(3)]
                for mt in range(NT):
                    for ch in range(3):
                        nc.tensor.matmul(
                            o_ts[ch],
                            wt_tiles[mt][:, ts(ntl, P)],
                            xb_tiles[mt][:, ts(ch, 512)],
                            start=(mt == 0),
                            stop=(mt == NT - 1),
                        )
                osb = osbp.tile([P, CT], F32, name="osb")
                for ch in range(3):
                    nc.scalar.activation(
                        out=osb[:, ts(ch, 512)],
                        in_=o_ts[ch],
                        func=mybir.ActivationFunctionType.Copy,
                        scale=recip_t[:, ntl : ntl + 1],
                    )
                nc.gpsimd.dma_start(
                    out=out_b[nt_g], in_=osb.rearrange("p (c t) -> p c t", t=T)
                )

        # Triangular schedule: phase1(h0) for m-tile j only needs k-tiles
        # 0..7 (the h0 n-range) plus tile j itself, so it interleaves with
        # the second half of the load stream and fills prologue PE idle.
        for mt in range(NT // 2):
            load_tile(mt)
        kgt_half(0)
        dn0 = ps_dn.tile([2, 512], F32, name="dn")
        wt0 = []
        for j in range(NT // 2):
            load_tile(NT // 2 + j)
            phase1_tile(0, j, dn0, wt0)
        for j in range(NT // 2, NT):
            phase1_tile(0, j, dn0, wt0)
        kgt_half(1)
        recip0 = recip_half(dn0)
        agg_half(0, wt0, recip0)

        dn1 = ps_dn.tile([2, 512], F32, name="dn")
        wt1 = []
        for mt in range(NT):
            phase1_tile(1, mt, dn1, wt1)
        recip1 = recip_half(dn1)
        agg_half(1, wt1, recip1)

    for p_ in reversed(ctx_pools):
        p_.release()


@functools.lru_cache(maxsize=4)
def _build_nc(reps=1):
    nc = bacc.Bacc(trn_type="TRN2")
    x = nc.dram_tensor("x", [BPC, C, N, T], F32, kind="ExternalInput")
    adjt = nc.dram_tensor("adjt", [N, N], F32, kind="ExternalInput")
    gw = nc.dram_tensor("gw", [T, T], F32, kind="ExternalInput")
    alpha = nc.dram_tensor("alpha", [C], F32, kind="ExternalInput")
    out = nc.dram_tensor("out", [BPC, C, N, T], F32, kind="ExternalOutput")
    with tile.TileContext(nc) as tc:
        _build_kernel_body(tc, x[:], adjt[:], gw[:], alpha[:], out[:], reps=reps)
    nc.finalize()
    return nc


def run(x, adj, Gw, alpha, trace=False):
    nc = _build_nc()
    x = np.ascontiguousarray(x, dtype=np.float32)
    adjt = np.ascontiguousarray(np.asarray(adj, dtype=np.float32).T)
    gw = np.ascontiguousarray(Gw, dtype=np.float32)
    al = np.ascontiguousarray(alpha, dtype=np.float32)
    in_maps = [
        {"x": x[i * BPC : (i + 1) * BPC], "adjt": adjt, "gw": gw, "alpha": al}
        for i in range(NCORES)
    ]
    res = run_bass_kernel_spmd(nc, in_maps, list(range(NCORES)), trace=trace)
    outv = np.concatenate([r["out"] for r in res.results], axis=0)
    return outv, res


def kernel(x, adj, Gw, alpha):
    outv, _ = run(x, adj, Gw, alpha, trace=False)
    return outv


# revision 28
# speedup vs baseline: 223.5497x; 223.5497x over previous
"""Trainium2 Bass kernel for nn_GAttention (gnn_message_passing).

Computation (per batch b):
    k  = einsum('cnt,c->nt', x[b], alpha)
    kG = k @ Gw
    S  = kG @ k.T                  # [N, N]
    att = softmax(S, axis=-1)      # rows
    out[b] = einsum('nm,cmt->cnt', att * adj, x[b])

Sharding: data-parallel over batch B=16 across 8 cores (2 batches/core).
adj/Gw/alpha replicated. No collectives.

Device-side layout strategy (per batch):
  - x loaded transposed as xT[m, (c,t)] tiles (m on partitions) via strided
    DMA (96B runs, split across both HWDGE rings in the prologue); cast to
    bf16 for the aggregation matmul; k computed from the fp32 tiles on DVE
    (alpha-weighted tree reduction over c into a 1-buf scratch).
  - scores computed TRANSPOSED: ST[m, n] = k[m] . kG[n] (kT/kGT in
    float32r for single-pass PE multiply) so the softmax-masked weights
    come out in the [m, n] layout the aggregation matmul needs as its
    stationary operand (contraction over m).
  - softmax without max-subtraction (scores are O(+-15), exp is safe in
    fp32); denominator = column sums of exp(ST) via a one-hot-stationary
    matmul accumulated into a single PSUM bank, reciprocal scattered to a
    per-partition [128, 1] layout with tiny DMAs.
  - adj^T (host-transposed input) is cast to bf16 once into an HBM scratch
    (lazily during batch 0's phase 1) and streamed per (batch, n-half).
  - aggregation: out2[n, (c,t)] = sum_m wT[m, n] * xT[m, (c,t)] in bf16,
    PSUM fp32 accumulation over 16 m-tiles; evicted through ScalarE with
    per-partition 1/denom scale, stored back with strided SWDGE DMA.
  - triangular schedule: a phase-1 unit (q, mt) needs only kGT chunk q
    (k-tiles 4q..4q+3) and k-tile mt, so most of phase 1 of the first
    n-half interleaves with the x-load stream, filling prologue PE idle;
    generous xb/wt pool bufs pipeline across halves and batches.

Cost-model (CoreSim no-exec, HW-calibrated): ~471 us/core; PE busy
~388 us (82%). End-to-end relative error vs fp32 reference: 4.8e-3.
"""

import functools

import numpy as np

import concourse.bass as bass
import concourse.bacc as bacc
import concourse.mybir as mybir
import concourse.tile as tile
from concourse.bass_utils import run_bass_kernel_spmd
from concourse.masks import make_identity

# Problem shape (hardcoded per contract).
B, C, N, T = 16, 64, 2048, 24
NCORES = 8
BPC = B // NCORES            # batches per core
P = 128                      # partitions
CT = C * T                   # 1536
NT = N // P                  # 16 n/m tiles
NHALF = 1024                 # n processed in halves (SBUF budget for wT)
NTL = NHALF // P             # 8 n-tiles per half
F32 = mybir.dt.float32
F32R = mybir.dt.float32r     # fp32 storage, single-pass PE multiply
BF16 = mybir.dt.bfloat16


def ts(i, sz):
    return bass.ts(i, sz)


def _build_kernel_body(tc: tile.TileContext, x, adjt, gw, alpha, out, reps=1):
    nc = tc.nc
    ctx_pools = []

    def pool(name, bufs, space="SBUF"):
        p = tc.alloc_tile_pool(name=name, bufs=bufs, space=space)
        ctx_pools.append(p)
        return p

    singles = pool("singles", 1)
    adjp = pool("adjp", 4)           # streamed bf16 adjT tiles (+ prepass)
    xfp = pool("xf", 3)              # fp32 x staging (strided loads land here)
    xbp = pool("xb", 24)             # bf16 xT tiles (16/batch + prefetch)
    kp = pool("kp", 2)               # k [128, 16, 24] per batch
    ktp = pool("ktp", 1)             # kT [24, 2048] f32r per batch
    kgp = pool("kgp", 1)             # kGT [24, 2048] f32r per batch
    ep = pool("ep", 12)              # exp(ST) bf16 chunks
    wtp = pool("wtp", 20)            # wT half-tiles [128, 1024] bf16
    osbp = pool("osb", 2)            # output staging fp32
    scrp = pool("scr", 1)            # k-chain scratch (DVE-serial, 1 buf ok)
    rcp = pool("rcp", 2)             # reciprocal denominators
    drp = pool("dram", 1, space="DRAM")      # bf16 adjT scratch in HBM
    ps_st = pool("ps_st", 2, space="PSUM")   # scores / small matmuls
    ps_dn = pool("ps_dn", 1, space="PSUM")   # denominator bank
    ps_o = pool("ps_o", 5, space="PSUM")     # aggregation accumulators

    # --- one-time setup ---------------------------------------------------
    ident = singles.tile([P, P], F32)
    make_identity(nc, ident)

    alpha_rep = singles.tile([P, C], F32)
    nc.gpsimd.dma_start(
        out=alpha_rep,
        in_=bass.AP(tensor=alpha.tensor, offset=0, ap=[[0, P], [1, C]]),
    )

    gw_sb = singles.tile([T, T], F32R)
    nc.gpsimd.dma_start(out=gw_sb, in_=gw[:, :])

    # e_q stationaries for the denominator trick: column q is ones.
    eq_tiles = []
    for q in range(2):
        e_q = singles.tile([P, 2], BF16, name=f"eq{q}")
        nc.vector.memset(e_q, 0.0)
        nc.vector.memset(e_q[:, q : q + 1], 1.0)
        eq_tiles.append(e_q)

    # adjT bf16 scratch in HBM: filled lazily on first use (batch 0's
    # phase 1), so the prologue's x-load stream gets full DMA bandwidth.
    adjt16 = drp.tile([N, N], BF16, name="adjt16")
    adjt16_filled = set()

    # --- per batch --------------------------------------------------------
    batch_seen = False
    for b in [bi for _ in range(reps) for bi in range(BPC)]:
        x_b = x[b].rearrange("c (mo p) t -> mo p c t", p=P)      # [16,128,C,T]
        out_b = out[b].rearrange("c (no p) t -> no p c t", p=P)  # [16,128,C,T]

        xb_tiles = []
        k_all = kp.tile([P, NT, T], F32, name="k_all")
        kt_sb = ktp.tile([T, N], F32R, name="kt")
        kgt_sb = kgp.tile([T, N], F32R, name="kgt")

        def load_tile(mt, x_b=x_b, k_all=k_all, kt_sb=kt_sb, xb_tiles=xb_tiles,
                      first=(b == 0)):
            """Strided xT load + k-chain + kT transpose + bf16 cast."""
            xf = xfp.tile([P, CT], F32, name="xf")
            xf3 = xf.rearrange("p (c t) -> p c t", t=T)
            if first and not batch_seen:
                # Prologue: split across both HWDGE rings (SP + ACT) so
                # descriptor generation for the 96B-chunk strided pattern
                # runs in parallel while ACT is otherwise idle.
                nc.sync.dma_start(
                    out=xf3[:, : C // 2, :], in_=x_b[mt][:, : C // 2, :]
                )
                nc.scalar.dma_start(
                    out=xf3[:, C // 2 :, :], in_=x_b[mt][:, C // 2 :, :]
                )
            else:
                # Steady state: keep the ACT sequencer free for exp/evict.
                nc.sync.dma_start(out=xf3, in_=x_b[mt])

            # k[m, t] = sum_c alpha[c] * x[c, m, t] via scratch so the
            # k-chain does not block on xb slot availability (batch overlap).
            scr = scrp.tile([P, C, T], F32, name="scr")
            nc.vector.tensor_tensor(
                scr,
                xf3,
                alpha_rep[:, :, None].to_broadcast((P, C, T)),
                mybir.AluOpType.mult,
            )
            s = C // 2
            while s >= 1:
                nc.vector.tensor_add(
                    out=scr[:, :s, :], in0=scr[:, :s, :], in1=scr[:, s : 2 * s, :]
                )
                s //= 2
            nc.vector.tensor_copy(out=k_all[:, mt, :], in_=scr[:, 0, :])

            # kT via PE transpose, interleaved with the load stream.
            ps = ps_st.tile([P, 512], F32, name="st")
            nc.tensor.transpose(ps[:T, :P], k_all[:, mt, :], ident)
            nc.vector.tensor_copy(out=kt_sb[:, ts(mt, P)], in_=ps[:T, :P])

            xb_t = xbp.tile([P, CT], BF16, name="xb")
            nc.vector.tensor_copy(out=xb_t, in_=xf)
            xb_tiles.append(xb_t)

        def kgt_q(qg, kt_sb=kt_sb, kgt_sb=kgt_sb):
            # kGT[s, n] = sum_t Gw[t, s] * kT[t, n], one 512-col chunk
            ps = ps_st.tile([P, 512], F32, name="st")
            nc.tensor.matmul(
                ps[:T, :512], gw_sb, kt_sb[:, ts(qg, 512)],
                start=True, stop=True,
            )
            nc.vector.tensor_copy(out=kgt_sb[:, ts(qg, 512)], in_=ps[:T, :512])

        # per-half phase-1 state: wt/adj tiles and the dn start/stop flags
        p1_state = {}

        def phase1_unit(h, q, mt, kt_sb=kt_sb, kgt_sb=kgt_sb):
            """ST -> exp -> denominator MM -> masked wT for one (m-tile, q)."""
            st = dn_state(h)
            if st["dn"] is None:
                st["dn"] = ps_dn.tile([2, 512], F32, name="dn")
            if mt not in st["adj"]:
                adj_t = adjp.tile([P, NHALF], BF16, name="adjs", tag="adj")
                if (h, mt) not in adjt16_filled:
                    # First touch ever: cast-load fp32 adjT and spill bf16
                    # copy to the HBM scratch for later batches.
                    adjt16_filled.add((h, mt))
                    nc.gpsimd.dma_start(
                        out=adj_t,
                        in_=adjt[ts(mt, P), h * NHALF : (h + 1) * NHALF],
                    )
                    nc.sync.dma_start(
                        out=adjt16[ts(mt, P), h * NHALF : (h + 1) * NHALF],
                        in_=adj_t,
                    )
                else:
                    nc.sync.dma_start(
                        out=adj_t,
                        in_=adjt16[ts(mt, P), h * NHALF : (h + 1) * NHALF],
                    )
                st["adj"][mt] = adj_t
                st["wt"][mt] = wtp.tile([P, NHALF], BF16, name="wt")
            adj_t = st["adj"][mt]
            wt_t = st["wt"][mt]

            nsl = slice(h * NHALF + q * 512, h * NHALF + (q + 1) * 512)
            st_t = ps_st.tile([P, 512], F32, name="st")
            nc.tensor.matmul(
                st_t, kt_sb[:, ts(mt, P)], kgt_sb[:, nsl],
                start=True, stop=True,
            )
            e_t = ep.tile([P, 512], BF16, name="e")
            nc.scalar.activation(
                out=e_t, in_=st_t, func=mybir.ActivationFunctionType.Exp
            )
            nunits = len(st["done"])
            nc.tensor.matmul(
                st["dn"], eq_tiles[q], e_t,
                start=(nunits == 0),
                stop=(nunits == 2 * NT - 1),
            )
            st["done"].add((q, mt))
            nc.vector.tensor_mul(
                out=wt_t[:, ts(q, 512)], in0=e_t, in1=adj_t[:, ts(q, 512)]
            )

        def recip_half(dn):
            # recipT[p, j] = 1 / denom[n = h*1024 + j*128 + p], j = 0..7
            recip_sb = rcp.tile([2, 512], F32, name="recip")
            nc.vector.reciprocal(out=recip_sb, in_=dn)
            recip_t = rcp.tile([P, NTL], F32, name="recipt")
            for j in range(NTL):
                q, j4 = divmod(j, 4)
                nc.sync.dma_start(
                    out=recip_t[:, j : j + 1],
                    in_=recip_sb[q : q + 1, j4 * P : (j4 + 1) * P],
                )
            return recip_t

        def agg_half(h, wt_tiles, recip_t, out_b=out_b, xb_tiles=xb_tiles,
                     last=False):
            for ntl in range(NTL):
                nt_g = h * NTL + ntl
                o_ts = [ps_o.tile([P, 512], F32, name="o") for _ in range(3)]
                for mt in range(NT):
                    for ch in range(3):
                        nc.tensor.matmul(
                            o_ts[ch],
                            wt_tiles[mt][:, ts(ntl, P)],
                            xb_tiles[mt][:, ts(ch, 512)],
                            start=(mt == 0),
                            stop=(mt == NT - 1),
                        )
                osb = osbp.tile([P, CT], F32, name="osb")
                for ch in range(3):
                    nc.scalar.activation(
                        out=osb[:, ts(ch, 512)],
                        in_=o_ts[ch],
                        func=mybir.ActivationFunctionType.Copy,
                        scale=recip_t[:, ntl : ntl + 1],
                    )
                osb3 = osb.rearrange("p (c t) -> p c t", t=T)
                if last and ntl == NTL - 1:
                    # Shorten the kernel tail: split the final store across
                    # the SWDGE and HWDGE rings so generation overlaps.
                    nc.gpsimd.dma_start(
                        out=out_b[nt_g][:, : C // 2, :], in_=osb3[:, : C // 2, :]
                    )
                    nc.sync.dma_start(
                        out=out_b[nt_g][:, C // 2 :, :], in_=osb3[:, C // 2 :, :]
                    )
                else:
                    nc.gpsimd.dma_start(out=out_b[nt_g], in_=osb3)

        # Triangular schedule: a phase-1 unit (q, mt) needs kGT chunk q
        # (= k-tiles 4q..4q+3) and k-tile mt only, so most of phase1(h0)
        # interleaves with the x-load stream and fills prologue PE idle.
        p1_state.clear()

        def ready_h0_units(l, limit):
            st0 = dn_state(0)
            n = 0
            for qg in range(2):
                if l < 4 * qg + 3:
                    continue
                for mt in range(NT):
                    if n == limit:
                        return
                    if mt > l or (qg, mt) in st0["done"]:
                        continue
                    yield (qg, mt)
                    n += 1

        def dn_state(h):
            return p1_state.setdefault(
                h, {"adj": {}, "wt": {}, "done": set(), "dn": None}
            )

        for l in range(NT):
            load_tile(l)
            if l in (3, 7, 11, 15):
                kgt_q((l - 3) // 4)
            if l >= 4:
                for qg, mt in list(ready_h0_units(l, 3)):
                    phase1_unit(0, qg, mt)
        st0 = dn_state(0)
        for qg in range(2):
            for mt in range(NT):
                if (qg, mt) not in st0["done"]:
                    phase1_unit(0, qg, mt)
        recip0 = recip_half(st0["dn"])
        agg_half(0, [st0["wt"][mt] for mt in range(NT)], recip0)

        for mt in range(NT):
            for qg in range(2):
                phase1_unit(1, qg, mt)
        st1 = p1_state[1]
        recip1 = recip_half(st1["dn"])
        agg_half(1, [st1["wt"][mt] for mt in range(NT)], recip1,
                 last=(b == BPC - 1))
        batch_seen = True

    for p_ in reversed(ctx_pools):
        p_.release()


@functools.lru_cache(maxsize=4)
def _build_nc(reps=1):
    nc = bacc.Bacc(trn_type="TRN2")
    x = nc.dram_tensor("x", [BPC, C, N, T], F32, kind="ExternalInput")
    adjt = nc.dram_tensor("adjt", [N, N], F32, kind="ExternalInput")
    gw = nc.dram_tensor("gw", [T, T], F32, kind="ExternalInput")
    alpha = nc.dram_tensor("alpha", [C], F32, kind="ExternalInput")
    out = nc.dram_tensor("out", [BPC, C, N, T], F32, kind="ExternalOutput")
    with tile.TileContext(nc) as tc:
        _build_kernel_body(tc, x[:], adjt[:], gw[:], alpha[:], out[:], reps=reps)
    nc.finalize()
    return nc


def run(x, adj, Gw, alpha, trace=False):
    nc = _build_nc()
    x = np.ascontiguousarray(x, dtype=np.float32)
    adjt = np.ascontiguousarray(np.asarray(adj, dtype=np.float32).T)
    gw = np.ascontiguousarray(Gw, dtype=np.float32)
    al = np.ascontiguousarray(alpha, dtype=np.float32)
    in_maps = [
        {"x": x[i * BPC : (i + 1) * BPC], "adjt": adjt, "gw": gw, "alpha": al}
        for i in range(NCORES)
    ]
    res = run_bass_kernel_spmd(nc, in_maps, list(range(NCORES)), trace=trace)
    outv = np.concatenate([r["out"] for r in res.results], axis=0)
    return outv, res


def kernel(x, adj, Gw, alpha):
    outv, _ = run(x, adj, Gw, alpha, trace=False)
    return outv
